# revision 1
# baseline (speedup 1.0000x reference)
"""Trainium2 Bass kernel for nn_BoxModel: box-embedding decode + log_softmax.

decoded[b, v] = sum_d log(softplus(min(cZ[b,d], vZ[v,d]) - max(cz[b,d], vz[v,d]))
                          + tiny) + bias[v]
out = log_softmax(decoded, axis=1)

Sharding: vocab axis split across 8 NeuronCores (4000 words each). Each core
computes its (64, 4000) slice of decoded plus a local logsumexp; one AllGather
of the 8x64 local LSEs gives every core the identical global LSE; host concats
the 8 output slices.

Key transformation: exp(min(cZ,vZ) - max(cz,vz)) = min(eVZ, c1) * min(eVZn, c2)
with eVZ = exp(vZ), eVZn = exp(-vz) precomputed per-vocab-shard (resident in
SBUF, [d=128 partitions, v free] layout) and c1 = exp(cZ[b]), c2 = exp(-cz[b])
per-partition scalars. That keeps the hot loop at 2 tensor_scalar (2x fp32) +
1 tensor_tensor on DVE and 2 Ln passes on ScalarE, with the cross-d reduction
done by one-hot-column matmuls on TensorE accumulating all 64 batch rows into
a bank-aligned PSUM tile.
"""

import sys

if "/opt/trn_rl_repo" not in sys.path:
    sys.path.insert(0, "/opt/trn_rl_repo")

import dataclasses

import numpy as np

import concourse.bass as bass
import concourse.bacc as bacc
import concourse.tile as tile
from concourse import mybir
from concourse.bass_utils import run_bass_kernel_spmd

VOCAB = 32000
DIM = 128
BATCH = 64
NGRAM = 4
NCORES = 8
VS = VOCAB // NCORES          # 4000 vocab words per core
HALF = VS // 2                # 2000
NBATCH = VS // 500            # 8 batches of 500 cols in resident precompute
CHUNK = 125                   # vocab rows per transpose chunk (4 per 500 batch)

F32 = mybir.dt.float32
I32 = mybir.dt.int32
AF = mybir.ActivationFunctionType
ALU = mybir.AluOpType
AX = mybir.AxisListType

_cache = {}


def _emit(nc, tc, aps, dbg=None):
    wb_full, wb_shard, xidx, bias_d, ident_d, sel_d, emat_d, out_d = aps
    v = nc.vector
    s = nc.scalar
    te = nc.tensor

    import contextlib

    ctx = contextlib.ExitStack()
    with ctx:
        consts = ctx.enter_context(tc.tile_pool(name="consts", bufs=1))
        resid = ctx.enter_context(tc.tile_pool(name="resid", bufs=1))
        work = ctx.enter_context(tc.tile_pool(name="work", bufs=2))
        psum = ctx.enter_context(tc.tile_pool(name="psum", bufs=1, space="PSUM"))
        dram = ctx.enter_context(tc.tile_pool(name="dram", bufs=1, space="DRAM"))

        # ---- constants ----
        ident = consts.tile([128, 128], F32, tag="ident")
        nc.sync.dma_start(out=ident[:], in_=ident_d[:])
        sel = consts.tile([128, 128], F32, tag="sel")
        nc.sync.dma_start(out=sel[:], in_=sel_d[:])
        idx0 = consts.tile([128, 1], I32, tag="idx0")
        nc.sync.dma_start(out=idx0[:], in_=xidx[0:128, :])
        idx1 = consts.tile([128, 1], I32, tag="idx1")
        nc.sync.dma_start(out=idx1[:], in_=xidx[128:256, :])

        # ---- context boxes: gather 256 rows, mean via selection matmul ----
        g0 = consts.tile([128, 2 * DIM], F32, tag="g0")
        nc.gpsimd.indirect_dma_start(
            out=g0[:], out_offset=None, in_=wb_full[:],
            in_offset=bass.IndirectOffsetOnAxis(ap=idx0[:, :1], axis=0),
        )
        g1 = consts.tile([128, 2 * DIM], F32, tag="g1")
        nc.gpsimd.indirect_dma_start(
            out=g1[:], out_offset=None, in_=wb_full[:],
            in_offset=bass.IndirectOffsetOnAxis(ap=idx1[:, :1], axis=0),
        )
        ctx_ps = psum.tile([64, 2 * DIM], F32, tag="dec")  # borrow dec's bank slot
        te.matmul(ctx_ps[:], lhsT=sel[:, 0:64], rhs=g0[:], start=True, stop=False)
        te.matmul(ctx_ps[:], lhsT=sel[:, 64:128], rhs=g1[:], start=False, stop=True)
        ctx_sb = consts.tile([64, 2 * DIM], F32, tag="ctx_sb")
        v.tensor_copy(ctx_sb[:], ctx_ps[:])

        # transpose ctx halves to [d, b]; compute c1 = exp(cZ), c2 = exp(-cz)
        czT_ps = psum.tile([128, 64], F32, tag="zT", bufs=2)
        te.transpose(czT_ps[:], ctx_sb[:, 0:DIM], ident[0:64, 0:64])
        cdT_ps = psum.tile([128, 64], F32, tag="dT", bufs=2)
        te.transpose(cdT_ps[:], ctx_sb[:, DIM:2 * DIM], ident[0:64, 0:64])

        c2 = consts.tile([128, 64], F32, tag="c2")
        s.activation(c2[:], czT_ps[:], AF.Exp, scale=-1.0)        # exp(-cz)
        t1 = consts.tile([128, 64], F32, tag="t1")
        s.activation(t1[:], cdT_ps[:], AF.Exp, scale=10.0)        # exp(10*cd)
        t2 = consts.tile([128, 64], F32, tag="t2")
        s.activation(t2[:], t1[:], AF.Ln, bias=1.0)               # softplus(10*cd)
        t3 = consts.tile([128, 64], F32, tag="t3")
        v.tensor_scalar_mul(t3[:], t2[:], 0.1)
        cZT = consts.tile([128, 64], F32, tag="cZT")
        v.tensor_tensor(out=cZT[:], in0=t3[:], in1=czT_ps[:], op=ALU.add)
        c1 = consts.tile([128, 64], F32, tag="c1")
        s.activation(c1[:], cZT[:], AF.Exp)                       # exp(cZ)

        # ---- resident vocab shard: eVZ = exp(vZ), eVZn = exp(-vz), [d, v] ----
        eVZ = [resid.tile([128, HALF], F32, tag=f"eVZ{h}", name=f"eVZ{h}")
               for h in range(2)]
        eVZn = [resid.tile([128, HALF], F32, tag=f"eVZn{h}", name=f"eVZn{h}")
                for h in range(2)]

        def emit_resident_batch(j):       # one batch of 500 vocab rows
            h, jcol = divmod(j * 500, HALF)
            zT = psum.tile([128, 500], F32, tag="zT", bufs=2, name=f"zT{j}")
            dT = psum.tile([128, 500], F32, tag="dT", bufs=2, name=f"dT{j}")
            for c in range(4):            # 125-row transpose chunks
                r0 = j * 500 + c * CHUNK
                zdn = work.tile([CHUNK, 2 * DIM], F32, tag="zdn", bufs=6,
                                name=f"zdn{j}_{c}")
                nc.sync.dma_start(out=zdn[:], in_=wb_shard[r0:r0 + CHUNK, :])
                cs = slice(c * CHUNK, (c + 1) * CHUNK)
                te.transpose(zT[:, cs], zdn[:, 0:DIM], ident[0:CHUNK, 0:CHUNK])
                te.transpose(dT[:, cs], zdn[:, DIM:2 * DIM], ident[0:CHUNK, 0:CHUNK])
            cols = slice(jcol, jcol + 500)
            # ACT order: u1, u2 (critical chain), then eVZn as ACT filler
            # while DVE runs u3/u4, then eVZ — minimizes ACT FIFO stalls
            u1 = work.tile([128, 500], F32, tag="u1", bufs=2, name=f"u1_{j}")
            s.activation(u1[:], dT[:], AF.Exp, scale=10.0)
            u2 = work.tile([128, 500], F32, tag="u2", bufs=2, name=f"u2_{j}")
            s.activation(u2[:], u1[:], AF.Ln, bias=1.0)
            u3 = work.tile([128, 500], F32, tag="u3", bufs=2, name=f"u3_{j}")
            v.tensor_scalar_mul(u3[:], u2[:], 0.1)
            s.activation(eVZn[h][:, cols], zT[:], AF.Exp, scale=-1.0)
            u4 = work.tile([128, 500], F32, tag="u4", bufs=2, name=f"u4_{j}")
            v.tensor_tensor(out=u4[:], in0=u3[:], in1=zT[:], op=ALU.add)
            s.activation(eVZ[h][:, cols], u4[:], AF.Exp)

        for j in range(4):                # resident precompute for half 0
            emit_resident_batch(j)

        # consts only needed by the main loop — emitted after the resident
        # chunk DMAs so they sit behind them in the sync DGE queue
        emat = consts.tile([128, BATCH * BATCH], F32, tag="emat")
        nc.sync.dma_start(out=emat[:], in_=emat_d[:])
        bias_rep = consts.tile([64, VS], F32, tag="bias_rep")
        bias_src = dataclasses.replace(bias_d[:], ap=[[0, 64]] + list(bias_d[:].ap))
        nc.sync.dma_start(out=bias_rep[:], in_=bias_src)

        # ---- main loop: dec[b, v] = sum_d ln(ln(E + 1)), E = A*B ----
        dec_sb = resid.tile([64, VS], F32, tag="dec_sb")
        # matmul outputs must be contained in single 2KB PSUM banks: pad the
        # accumulator to 2048 floats (4 banks) and slice 512/512/512/464
        qb = [(0, 512), (512, 1024), (1024, 1536), (1536, 2000)]
        for h in range(2):
            if h == 1:
                for j in range(4, NBATCH):  # half-1 precompute overlaps main h0
                    emit_resident_batch(j)
            dec_ps = psum.tile([64, 2048], F32, tag="dec", name=f"dec{h}")
            for b in range(BATCH):
                A = work.tile([128, HALF], F32, tag="A")
                v.tensor_scalar_min(A[:], eVZ[h][:], c1[:, b:b + 1])
                B = work.tile([128, HALF], F32, tag="B")
                v.tensor_scalar_min(B[:], eVZn[h][:], c2[:, b:b + 1])
                E = work.tile([128, HALF], F32, tag="E")
                v.tensor_tensor(out=E[:], in0=A[:], in1=B[:], op=ALU.mult)
                s1 = work.tile([128, HALF], F32, tag="s1", name=f"s1_{h}_{b}")
                s.activation(s1[:], E[:], AF.Ln, bias=1.0)        # ln(E+1) = side
                ls = work.tile([128, HALF], F32, tag="ls", bufs=2,
                               name=f"ls_{h}_{b}")
                s.activation(ls[:], s1[:], AF.Ln, bias=0.0)       # ln(side)
                for q0, q1 in qb:
                    qs = slice(q0, q1)
                    te.matmul(dec_ps[:, qs],
                              lhsT=emat[:, b * BATCH:(b + 1) * BATCH],
                              rhs=ls[:, qs],
                              start=(b == 0), stop=(b == BATCH - 1))
            hs = slice(h * HALF, (h + 1) * HALF)
            v.tensor_tensor(out=dec_sb[:, hs], in0=dec_ps[:, 0:HALF],
                            in1=bias_rep[:, hs], op=ALU.add)

        # ---- local logsumexp over the 4000-col shard ----
        M = consts.tile([64, 1], F32, tag="M")
        v.reduce_max(out=M[:], in_=dec_sb[:], axis=AX.X)
        negM = consts.tile([64, 1], F32, tag="negM")
        v.tensor_scalar_mul(negM[:], M[:], -1.0)
        e2 = work.tile([64, VS], F32, tag="e2", bufs=1)
        S = consts.tile([64, 1], F32, tag="S")
        s.activation(e2[:], dec_sb[:], AF.Exp, bias=negM[:, 0:1], accum_out=S[:])
        lnS = consts.tile([64, 1], F32, tag="lnS")
        s.activation(lnS[:], S[:], AF.Ln)
        lse = consts.tile([64, 1], F32, tag="lse")
        v.tensor_tensor(out=lse[:], in0=M[:], in1=lnS[:], op=ALU.add)

        # ---- AllGather local LSEs -> identical global LSE everywhere ----
        cc_in = dram.tile([64, 1], F32, tag="cc_in")
        nc.sync.dma_start(out=cc_in[:], in_=lse[:])
        cc_out = dram.tile([NCORES * 64, 1], F32, tag="cc_out")
        nc.gpsimd.collective_compute(
            "AllGather", ALU.bypass,
            replica_groups=[list(range(NCORES))],
            ins=[cc_in[:].opt()], outs=[cc_out[:].opt()],
        )
        lse_all = consts.tile([64, NCORES], F32, tag="lse_all")
        src = dataclasses.replace(cc_out[:], ap=[[1, 64], [64, NCORES]])
        nc.sync.dma_start(out=lse_all[:], in_=src)

        M2 = consts.tile([64, 1], F32, tag="M2")
        v.reduce_max(out=M2[:], in_=lse_all[:], axis=AX.X)
        negM2 = consts.tile([64, 1], F32, tag="negM2")
        v.tensor_scalar_mul(negM2[:], M2[:], -1.0)
        e3 = consts.tile([64, NCORES], F32, tag="e3")
        S2 = consts.tile([64, 1], F32, tag="S2")
        s.activation(e3[:], lse_all[:], AF.Exp, bias=negM2[:, 0:1], accum_out=S2[:])
        lnS2 = consts.tile([64, 1], F32, tag="lnS2")
        s.activation(lnS2[:], S2[:], AF.Ln)
        G = consts.tile([64, 1], F32, tag="G")
        v.tensor_tensor(out=G[:], in0=M2[:], in1=lnS2[:], op=ALU.add)

        # ---- out = dec - G, store ----
        out_sb = work.tile([64, VS], F32, tag="e2", bufs=1)  # reuse e2's slot
        v.tensor_scalar(out=out_sb[:], in0=dec_sb[:], scalar1=G[:, 0:1],
                        scalar2=None, op0=ALU.subtract)
        nc.sync.dma_start(out=out_d[:], in_=out_sb[:])

        if dbg is not None:
            nc.sync.dma_start(out=dbg["ctx"][:], in_=ctx_sb[:])
            nc.sync.dma_start(out=dbg["c1"][:], in_=c1[:])
            nc.sync.dma_start(out=dbg["c2"][:], in_=c2[:])
            nc.sync.dma_start(out=dbg["eVZ"][:], in_=eVZ[0][:, 0:128])
            nc.sync.dma_start(out=dbg["eVZn"][:], in_=eVZn[0][:, 0:128])
            nc.sync.dma_start(out=dbg["dec"][:], in_=dec_sb[:])
            nc.sync.dma_start(out=dbg["lse_all"][:], in_=lse_all[:])
            nc.sync.dma_start(out=dbg["g0"][:], in_=g0[:])


def _build(debug=False):
    key = ("nc", debug)
    if key in _cache:
        return _cache[key]
    nc = bacc.Bacc("TRN2", target_bir_lowering=False, debug=False,
                   num_devices=NCORES)
    wb_full = nc.dram_tensor("wb_full", [VOCAB, 2 * DIM], F32,
                             kind="ExternalInput").ap()
    wb_shard = nc.dram_tensor("wb_shard", [VS, 2 * DIM], F32,
                              kind="ExternalInput").ap()
    xidx = nc.dram_tensor("xidx", [BATCH * NGRAM, 1], I32,
                          kind="ExternalInput").ap()
    bias_d = nc.dram_tensor("bias", [VS], F32, kind="ExternalInput").ap()
    ident_d = nc.dram_tensor("ident", [128, 128], F32, kind="ExternalInput").ap()
    sel_d = nc.dram_tensor("sel", [128, 128], F32, kind="ExternalInput").ap()
    emat_d = nc.dram_tensor("emat", [128, BATCH * BATCH], F32,
                            kind="ExternalInput").ap()
    out_d = nc.dram_tensor("out", [BATCH, VS], F32, kind="ExternalOutput").ap()
    dbg = None
    if debug:
        shapes = {"ctx": [64, 256], "c1": [128, 64], "c2": [128, 64],
                  "eVZ": [128, 128], "eVZn": [128, 128], "dec": [64, VS],
                  "lse_all": [64, 8], "g0": [128, 256]}
        dbg = {k: nc.dram_tensor(f"dbg_{k}", sh, F32, kind="ExternalOutput").ap()
               for k, sh in shapes.items()}

    with tile.TileContext(nc) as tc:
        _emit(nc, tc, (wb_full, wb_shard, xidx, bias_d, ident_d, sel_d, emat_d,
                       out_d), dbg=dbg)
    nc.compile()
    _cache[key] = nc
    return nc


def _consts():
    ident = np.eye(128, dtype=np.float32)
    sel = np.zeros((128, 128), dtype=np.float32)
    r = np.arange(128)
    sel[r, r // 4] = 0.25          # rows 0..127  -> b 0..31
    sel[r, 64 + 32 + r // 4] = 0.25  # rows 128..255 -> b 32..63 (second half)
    # emat[d, b*64 + m] = [m == b] : one-hot lhsT columns for the per-b
    # cross-partition sum matmul (same for every partition d)
    emat = np.tile(np.eye(BATCH, dtype=np.float32).reshape(1, -1), (128, 1))
    return ident, sel, emat


def _run(x, word_boxes, bias, trace=False, debug=False):
    nc = _build(debug=debug)
    ident, sel, emat = _consts()
    wbf = np.ascontiguousarray(
        np.asarray(word_boxes, dtype=np.float32).reshape(VOCAB, 2 * DIM))
    xf = np.ascontiguousarray(
        np.asarray(x).astype(np.int32).reshape(BATCH * NGRAM, 1))
    bias_f = np.asarray(bias, dtype=np.float32).reshape(VOCAB)
    in_maps = []
    for k in range(NCORES):
        vs = slice(k * VS, (k + 1) * VS)
        in_maps.append({
            "wb_full": wbf,
            "wb_shard": np.ascontiguousarray(wbf[vs]),
            "xidx": xf,
            "bias": np.ascontiguousarray(bias_f[vs]),
            "ident": ident,
            "sel": sel,
            "emat": emat,
        })
    res = run_bass_kernel_spmd(nc, in_maps, list(range(NCORES)), trace=trace)
    out = np.concatenate([res.results[k]["out"] for k in range(NCORES)], axis=1)
    return out, res


def kernel(x, word_boxes, bias):
    out, _ = _run(x, word_boxes, bias)
    return out

